# revision 4
# baseline (speedup 1.0000x reference)
"""ASTRA contrastive loss on 8 Trainium2 NeuronCores (Bass/Tile), v3.

Pure data parallel: batch B=1024 sharded 128 samples/core (one SBUF
partition per sample). HBM traffic halved by staging embeddings as
bfloat16 (host-side dtype cast only; final rel-err ~1e-4 vs 2e-2 budget).

Engine split per core (model, us):
    GpSimd  prod = mut*heal           ~36
    ACT     msq/hsq big-slab squares  ~31
    DVE     radix-2 fold trees over D on a merged [128,GA,3,D] tile
            (one instr/level covers all 3 stats, bf16 2x mode) + one
            merged tensor_reduce -> f32 stats + short Rsqrt epilogue ~34
    DMA     8.4 MiB loads             ~23

Notes: fp16 does NOT engage the DVE 2x uops on this hardware (measured
1x) -- bf16 required. tensor_tensor_reduce (custom ANT-DVE ucode)
crashes NRT. ACT accum_out per-agent costs ~2x big-slab square + folds.
"""

import sys

import numpy as np

_REPO = "/opt/trn_rl_repo"
if _REPO not in sys.path:
    sys.path.insert(0, _REPO)

B, N, D = 1024, 64, 256
NCORES = 8
BP = B // NCORES          # samples per core (one SBUF partition each)
GROUPS = 4                # DMA chunks per tensor per core
GA = N // GROUPS          # agents per chunk
MULT_ENGINE = "gpsimd"    # "dve" | "gpsimd": engine for the mut*heal product
FOLD_STOP = 32            # fold stats down to this width, then tensor_reduce
MARGIN = 1.0
ALPHA = 0.7
EPS = 1e-8

_NC_CACHE = {}


def _build_nc(reps=1):
    """Build the single-core Bass/Tile program (SPMD across 8 cores)."""
    from contextlib import ExitStack

    import concourse.bacc as bacc
    import concourse.tile as tile
    from concourse import mybir

    bf16 = mybir.dt.bfloat16
    f32 = mybir.dt.float32
    Alu = mybir.AluOpType
    Act = mybir.ActivationFunctionType

    nc = bacc.Bacc(None, target_bir_lowering=False, debug=False, num_devices=NCORES)
    mut_d = nc.declare_dram_parameter("emb_mut", [BP, N, D], bf16, isOutput=False)
    heal_d = nc.declare_dram_parameter("emb_heal", [BP, N, D], bf16, isOutput=False)
    idx_d = nc.declare_dram_parameter("idx_f", [BP, 1], f32, isOutput=False)
    mask_d = nc.declare_dram_parameter("mask_f", [BP, N], f32, isOutput=False)
    iota_d = nc.declare_dram_parameter("iota_f", [BP, N], f32, isOutput=False)
    out_d = nc.declare_dram_parameter("out", [BP, 2], f32, isOutput=True)

    def emit_body(tc, ctx, pools):
        (mut_pool, heal_pool, t3_pool, st_pool, ep_pool) = pools

        # stats: [sample, agent, {dot, ssm, ssh}] f32
        st3 = st_pool.tile([BP, N, 3], f32, tag="st3")

        # epilogue inputs are independent of the main loop: load them first
        idx_t = ep_pool.tile([BP, 1], f32, tag="idx")
        nc.sync.dma_start(out=idx_t[:, :], in_=idx_d[:, :])
        mask_t = ep_pool.tile([BP, N], f32, tag="mask")
        nc.sync.dma_start(out=mask_t[:, :], in_=mask_d[:, :])
        iota_t = ep_pool.tile([BP, N], f32, tag="iota")
        nc.sync.dma_start(out=iota_t[:, :], in_=iota_d[:, :])

        for g in range(GROUPS):
            gsl = slice(g * GA, (g + 1) * GA)
            mt = mut_pool.tile([BP, GA, D], bf16, tag="mt")
            nc.sync.dma_start(out=mt[:, :, :], in_=mut_d[:, gsl, :])
            ht = heal_pool.tile([BP, GA, D], bf16, tag="ht")
            nc.sync.dma_start(out=ht[:, :, :], in_=heal_d[:, gsl, :])

            # merged work tile: [:, :, 0]=prod, [:, :, 1]=mut^2, [:, :, 2]=heal^2
            t3 = t3_pool.tile([BP, GA, 3, D], bf16, tag="t3")
            if MULT_ENGINE == "gpsimd":
                nc.gpsimd.tensor_tensor(out=t3[:, :, 0, :], in0=mt[:, :, :],
                                        in1=ht[:, :, :], op=Alu.mult)
            else:
                nc.vector.tensor_tensor(out=t3[:, :, 0, :], in0=mt[:, :, :],
                                        in1=ht[:, :, :], op=Alu.mult)
            nc.scalar.activation(out=t3[:, :, 1, :], in_=mt[:, :, :],
                                 func=Act.Square)
            nc.scalar.activation(out=t3[:, :, 2, :], in_=ht[:, :, :],
                                 func=Act.Square)

            # DVE radix-2 fold tree along D, one instruction per level for
            # all three stats (bf16, unit stride -> 2x mode)
            w = D // 2
            while w >= FOLD_STOP:
                nc.vector.tensor_tensor(out=t3[:, :, :, 0:w],
                                        in0=t3[:, :, :, 0:w],
                                        in1=t3[:, :, :, w:2 * w], op=Alu.add)
                w //= 2
            nc.vector.tensor_reduce(out=st3[:, gsl, :],
                                    in_=t3[:, :, :, 0:FOLD_STOP],
                                    axis=mybir.AxisListType.X, op=Alu.add)

        dot = st3[:, :, 0]
        ssm = st3[:, :, 1]
        ssh = st3[:, :, 2]

        # ---- tiny per-sample epilogue, all [128, 64] / [128, 1] f32 ----
        # cos = dot * rsqrt(max(ssm*ssh, eps^4))
        den2 = ep_pool.tile([BP, N], f32, tag="den2")
        nc.vector.tensor_tensor(out=den2[:, :], in0=ssm, in1=ssh, op=Alu.mult)
        nc.vector.tensor_scalar(out=den2[:, :], in0=den2[:, :],
                                scalar1=EPS * EPS * EPS * EPS, scalar2=None,
                                op0=Alu.max)
        den = ep_pool.tile([BP, N], f32, tag="den")
        nc.scalar.activation(out=den[:, :], in_=den2[:, :], func=Act.Sqrt)
        rden = ep_pool.tile([BP, N], f32, tag="rden")
        nc.vector.reciprocal(out=rden[:, :], in_=den[:, :])
        cos = ep_pool.tile([BP, N], f32, tag="cos")
        nc.vector.tensor_tensor(out=cos[:, :], in0=dot, in1=rden[:, :],
                                op=Alu.mult)

        # validity and clipped index
        v0 = ep_pool.tile([BP, 1], f32, tag="v0")
        nc.vector.tensor_scalar(out=v0[:, :], in0=idx_t[:, :], scalar1=0.0,
                                scalar2=None, op0=Alu.is_ge)
        v1 = ep_pool.tile([BP, 1], f32, tag="v1")
        nc.vector.tensor_scalar(out=v1[:, :], in0=idx_t[:, :], scalar1=float(N),
                                scalar2=None, op0=Alu.is_lt)
        valid = ep_pool.tile([BP, 1], f32, tag="valid")
        nc.vector.tensor_tensor(out=valid[:, :], in0=v0[:, :], in1=v1[:, :],
                                op=Alu.mult)
        idx_c = ep_pool.tile([BP, 1], f32, tag="idxc")
        nc.vector.tensor_scalar(out=idx_c[:, :], in0=idx_t[:, :], scalar1=0.0,
                                scalar2=float(N - 1), op0=Alu.max, op1=Alu.min)

        # one-hot of target agent; cos at target
        onehot = ep_pool.tile([BP, N], f32, tag="onehot")
        nc.vector.tensor_scalar(out=onehot[:, :], in0=iota_t[:, :],
                                scalar1=idx_c[:, 0:1], scalar2=None,
                                op0=Alu.is_equal)
        ct_prod = ep_pool.tile([BP, N], f32, tag="ctprod")
        nc.vector.tensor_tensor(out=ct_prod[:, :], in0=cos[:, :],
                                in1=onehot[:, :], op=Alu.mult)
        cos_t = ep_pool.tile([BP, 1], f32, tag="cost")
        nc.vector.tensor_reduce(out=cos_t[:, :], in_=ct_prod[:, :],
                                axis=mybir.AxisListType.X, op=Alu.add)
        loss_t = ep_pool.tile([BP, 1], f32, tag="losst")
        nc.vector.tensor_scalar(out=loss_t[:, :], in0=cos_t[:, :],
                                scalar1=MARGIN, scalar2=0.0,
                                op0=Alu.add, op1=Alu.max)

        # normal mask, count, sum of (1-cos) over normal agents
        notT = ep_pool.tile([BP, N], f32, tag="notT")
        nc.vector.tensor_scalar(out=notT[:, :], in0=onehot[:, :], scalar1=-1.0,
                                scalar2=1.0, op0=Alu.mult, op1=Alu.add)
        nmask = ep_pool.tile([BP, N], f32, tag="nmask")
        nc.vector.tensor_tensor(out=nmask[:, :], in0=mask_t[:, :],
                                in1=notT[:, :], op=Alu.mult)
        cnt = ep_pool.tile([BP, 1], f32, tag="cnt")
        nc.vector.tensor_reduce(out=cnt[:, :], in_=nmask[:, :],
                                axis=mybir.AxisListType.X, op=Alu.add)
        mc_prod = ep_pool.tile([BP, N], f32, tag="mcprod")
        nc.vector.tensor_tensor(out=mc_prod[:, :], in0=nmask[:, :],
                                in1=cos[:, :], op=Alu.mult)
        mc = ep_pool.tile([BP, 1], f32, tag="mc")
        nc.vector.tensor_reduce(out=mc[:, :], in_=mc_prod[:, :],
                                axis=mybir.AxisListType.X, op=Alu.add)
        so = ep_pool.tile([BP, 1], f32, tag="so")
        nc.vector.tensor_tensor(out=so[:, :], in0=cnt[:, :], in1=mc[:, :],
                                op=Alu.subtract)

        # loss_others = (cnt>0) ? so / max(cnt,1) : 0
        cnt1 = ep_pool.tile([BP, 1], f32, tag="cnt1")
        nc.vector.tensor_scalar(out=cnt1[:, :], in0=cnt[:, :], scalar1=1.0,
                                scalar2=None, op0=Alu.max)
        icnt = ep_pool.tile([BP, 1], f32, tag="icnt")
        nc.vector.reciprocal(out=icnt[:, :], in_=cnt1[:, :])
        gpos = ep_pool.tile([BP, 1], f32, tag="gpos")
        nc.vector.tensor_scalar(out=gpos[:, :], in0=cnt[:, :], scalar1=0.0,
                                scalar2=None, op0=Alu.is_gt)
        lo = ep_pool.tile([BP, 1], f32, tag="lo")
        nc.vector.tensor_tensor(out=lo[:, :], in0=so[:, :], in1=icnt[:, :],
                                op=Alu.mult)
        nc.vector.tensor_tensor(out=lo[:, :], in0=lo[:, :], in1=gpos[:, :],
                                op=Alu.mult)

        # per-sample loss, gated by validity
        pa = ep_pool.tile([BP, 1], f32, tag="pa")
        nc.vector.tensor_scalar(out=pa[:, :], in0=loss_t[:, :], scalar1=ALPHA,
                                scalar2=None, op0=Alu.mult)
        pb = ep_pool.tile([BP, 1], f32, tag="pb")
        nc.vector.tensor_scalar(out=pb[:, :], in0=lo[:, :],
                                scalar1=1.0 - ALPHA, scalar2=None, op0=Alu.mult)
        per = ep_pool.tile([BP, 1], f32, tag="per")
        nc.vector.tensor_tensor(out=per[:, :], in0=pa[:, :], in1=pb[:, :],
                                op=Alu.add)
        contrib = ep_pool.tile([BP, 1], f32, tag="contrib")
        nc.vector.tensor_tensor(out=contrib[:, :], in0=per[:, :],
                                in1=valid[:, :], op=Alu.mult)

        out_sb = ep_pool.tile([BP, 2], f32, tag="outsb")
        nc.vector.tensor_copy(out_sb[:, 0:1], contrib[:, :])
        nc.vector.tensor_copy(out_sb[:, 1:2], valid[:, :])
        nc.sync.dma_start(out=out_d[:, :], in_=out_sb[:, :])

    with tile.TileContext(nc) as tc, ExitStack() as ctx:
        pools = (
            ctx.enter_context(tc.tile_pool(name="mut", bufs=GROUPS)),
            ctx.enter_context(tc.tile_pool(name="heal", bufs=GROUPS)),
            ctx.enter_context(tc.tile_pool(name="t3", bufs=2)),
            ctx.enter_context(tc.tile_pool(name="stats", bufs=2)),
            ctx.enter_context(tc.tile_pool(name="epi", bufs=2)),
        )
        if reps == 1:
            emit_body(tc, ctx, pools)
        else:
            with tc.For_i(0, reps, 1):
                emit_body(tc, ctx, pools)

    nc.compile()
    return nc


def _get_nc(reps=1):
    key = ("nc", reps)
    if key not in _NC_CACHE:
        _NC_CACHE[key] = _build_nc(reps)
    return _NC_CACHE[key]


def _make_in_maps(inputs):
    import ml_dtypes

    bf = ml_dtypes.bfloat16
    mut = np.ascontiguousarray(
        np.asarray(inputs["emb_mut"], dtype=np.float32).astype(bf))
    heal = np.ascontiguousarray(
        np.asarray(inputs["emb_heal"], dtype=np.float32).astype(bf))
    idx_f = np.asarray(inputs["mistake_agent_idx"]).astype(np.float32).reshape(B, 1)
    mask_f = np.asarray(inputs["agent_mask"]).astype(np.float32).reshape(B, N)
    iota_f = np.ascontiguousarray(
        np.broadcast_to(np.arange(N, dtype=np.float32), (BP, N))
    )
    in_maps = []
    for c in range(NCORES):
        sl = slice(c * BP, (c + 1) * BP)
        in_maps.append({
            "emb_mut": mut[sl],
            "emb_heal": heal[sl],
            "idx_f": np.ascontiguousarray(idx_f[sl]),
            "mask_f": np.ascontiguousarray(mask_f[sl]),
            "iota_f": iota_f,
        })
    return in_maps


def run_spmd(inputs, trace=False, reps=1):
    """Run on all 8 cores; returns (final_scalar, BassKernelResults)."""
    from concourse.bass_utils import run_bass_kernel_spmd

    nc = _get_nc(reps)
    in_maps = _make_in_maps(inputs)
    res = run_bass_kernel_spmd(nc, in_maps, list(range(NCORES)), trace=trace)
    outs = np.stack([r["out"] for r in res.results])  # [8, 128, 2]
    total = outs[..., 0].sum(dtype=np.float64)
    count = outs[..., 1].sum(dtype=np.float64)
    val = np.float32(total / count) if count > 0 else np.float32(0.0)
    return val, res


def kernel(**inputs) -> np.ndarray:
    val, _ = run_spmd(inputs, trace=False)
    return val


# revision 5
# speedup vs baseline: 1.2800x; 1.2800x over previous
"""ASTRA contrastive loss on 8 Trainium2 NeuronCores (Bass/Tile), v3.

Pure data parallel: batch B=1024 sharded 128 samples/core (one SBUF
partition per sample). HBM traffic halved by staging embeddings as
bfloat16 (host-side dtype cast only; final rel-err ~1e-4 vs 2e-2 budget).

Engine split per core (model, us):
    GpSimd  prod = mut*heal           ~36
    ACT     msq/hsq big-slab squares  ~31
    DVE     radix-2 fold trees over D on a merged [128,GA,3,D] tile
            (one instr/level covers all 3 stats, bf16 2x mode) + one
            merged tensor_reduce -> f32 stats + short Rsqrt epilogue ~34
    DMA     8.4 MiB loads             ~23

Notes: fp16 does NOT engage the DVE 2x uops on this hardware (measured
1x) -- bf16 required. tensor_tensor_reduce (custom ANT-DVE ucode)
crashes NRT. ACT accum_out per-agent costs ~2x big-slab square + folds.
"""

import sys

import numpy as np

_REPO = "/opt/trn_rl_repo"
if _REPO not in sys.path:
    sys.path.insert(0, _REPO)

B, N, D = 1024, 64, 256
NCORES = 8
BP = B // NCORES          # samples per core (one SBUF partition each)
GROUPS = 4                # DMA chunks per tensor per core
GA = N // GROUPS          # agents per chunk
MULT_ENGINE = "dve"    # "dve" | "gpsimd": engine for the mut*heal product
FOLD_STOP = 16            # fold stats down to this width, then tensor_reduce
MARGIN = 1.0
ALPHA = 0.7
EPS = 1e-8

_NC_CACHE = {}


def _build_nc(reps=1):
    """Build the single-core Bass/Tile program (SPMD across 8 cores)."""
    from contextlib import ExitStack

    import concourse.bacc as bacc
    import concourse.tile as tile
    from concourse import mybir

    bf16 = mybir.dt.bfloat16
    f32 = mybir.dt.float32
    Alu = mybir.AluOpType
    Act = mybir.ActivationFunctionType

    nc = bacc.Bacc(None, target_bir_lowering=False, debug=False, num_devices=NCORES)
    mut_d = nc.declare_dram_parameter("emb_mut", [BP, N, D], bf16, isOutput=False)
    heal_d = nc.declare_dram_parameter("emb_heal", [BP, N, D], bf16, isOutput=False)
    idx_d = nc.declare_dram_parameter("idx_f", [BP, 1], f32, isOutput=False)
    mask_d = nc.declare_dram_parameter("mask_f", [BP, N], f32, isOutput=False)
    iota_d = nc.declare_dram_parameter("iota_f", [BP, N], f32, isOutput=False)
    out_d = nc.declare_dram_parameter("out", [BP, 2], f32, isOutput=True)

    def emit_body(tc, ctx, pools):
        (mut_pool, heal_pool, t3_pool, st_pool, ep_pool) = pools

        # stats: [sample, agent, {dot, ssm, ssh}] f32
        st3 = st_pool.tile([BP, N, 3], f32, tag="st3")

        # epilogue inputs are independent of the main loop: load them first
        idx_t = ep_pool.tile([BP, 1], f32, tag="idx")
        nc.sync.dma_start(out=idx_t[:, :], in_=idx_d[:, :])
        mask_t = ep_pool.tile([BP, N], f32, tag="mask")
        nc.sync.dma_start(out=mask_t[:, :], in_=mask_d[:, :])
        iota_t = ep_pool.tile([BP, N], f32, tag="iota")
        nc.sync.dma_start(out=iota_t[:, :], in_=iota_d[:, :])

        for g in range(GROUPS):
            gsl = slice(g * GA, (g + 1) * GA)
            mt = mut_pool.tile([BP, GA, D], bf16, tag="mt")
            nc.sync.dma_start(out=mt[:, :, :], in_=mut_d[:, gsl, :])
            ht = heal_pool.tile([BP, GA, D], bf16, tag="ht")
            nc.sync.dma_start(out=ht[:, :, :], in_=heal_d[:, gsl, :])

            # merged work tile: [:, :, 0]=prod, [:, :, 1]=mut^2, [:, :, 2]=heal^2
            t3 = t3_pool.tile([BP, GA, 3, D], bf16, tag="t3")
            if MULT_ENGINE == "gpsimd":
                nc.gpsimd.tensor_tensor(out=t3[:, :, 0, :], in0=mt[:, :, :],
                                        in1=ht[:, :, :], op=Alu.mult)
            else:
                nc.vector.tensor_tensor(out=t3[:, :, 0, :], in0=mt[:, :, :],
                                        in1=ht[:, :, :], op=Alu.mult)
            nc.scalar.activation(out=t3[:, :, 1, :], in_=mt[:, :, :],
                                 func=Act.Square)
            nc.scalar.activation(out=t3[:, :, 2, :], in_=ht[:, :, :],
                                 func=Act.Square)

            # DVE radix-2 fold tree along D, one instruction per level for
            # all three stats (bf16, unit stride -> 2x mode)
            w = D // 2
            while w >= FOLD_STOP:
                nc.vector.tensor_tensor(out=t3[:, :, :, 0:w],
                                        in0=t3[:, :, :, 0:w],
                                        in1=t3[:, :, :, w:2 * w], op=Alu.add)
                w //= 2
            nc.vector.tensor_reduce(out=st3[:, gsl, :],
                                    in_=t3[:, :, :, 0:FOLD_STOP],
                                    axis=mybir.AxisListType.X, op=Alu.add)

        dot = st3[:, :, 0]
        ssm = st3[:, :, 1]
        ssh = st3[:, :, 2]

        # ---- tiny per-sample epilogue, all [128, 64] / [128, 1] f32 ----
        # cos = dot * rsqrt(max(ssm*ssh, eps^4))
        den2 = ep_pool.tile([BP, N], f32, tag="den2")
        nc.vector.tensor_tensor(out=den2[:, :], in0=ssm, in1=ssh, op=Alu.mult)
        nc.vector.tensor_scalar(out=den2[:, :], in0=den2[:, :],
                                scalar1=EPS * EPS * EPS * EPS, scalar2=None,
                                op0=Alu.max)
        den = ep_pool.tile([BP, N], f32, tag="den")
        nc.scalar.activation(out=den[:, :], in_=den2[:, :], func=Act.Sqrt)
        rden = ep_pool.tile([BP, N], f32, tag="rden")
        nc.vector.reciprocal(out=rden[:, :], in_=den[:, :])
        cos = ep_pool.tile([BP, N], f32, tag="cos")
        nc.vector.tensor_tensor(out=cos[:, :], in0=dot, in1=rden[:, :],
                                op=Alu.mult)

        # validity and clipped index
        v0 = ep_pool.tile([BP, 1], f32, tag="v0")
        nc.vector.tensor_scalar(out=v0[:, :], in0=idx_t[:, :], scalar1=0.0,
                                scalar2=None, op0=Alu.is_ge)
        v1 = ep_pool.tile([BP, 1], f32, tag="v1")
        nc.vector.tensor_scalar(out=v1[:, :], in0=idx_t[:, :], scalar1=float(N),
                                scalar2=None, op0=Alu.is_lt)
        valid = ep_pool.tile([BP, 1], f32, tag="valid")
        nc.vector.tensor_tensor(out=valid[:, :], in0=v0[:, :], in1=v1[:, :],
                                op=Alu.mult)
        idx_c = ep_pool.tile([BP, 1], f32, tag="idxc")
        nc.vector.tensor_scalar(out=idx_c[:, :], in0=idx_t[:, :], scalar1=0.0,
                                scalar2=float(N - 1), op0=Alu.max, op1=Alu.min)

        # one-hot of target agent; cos at target
        onehot = ep_pool.tile([BP, N], f32, tag="onehot")
        nc.vector.tensor_scalar(out=onehot[:, :], in0=iota_t[:, :],
                                scalar1=idx_c[:, 0:1], scalar2=None,
                                op0=Alu.is_equal)
        ct_prod = ep_pool.tile([BP, N], f32, tag="ctprod")
        nc.vector.tensor_tensor(out=ct_prod[:, :], in0=cos[:, :],
                                in1=onehot[:, :], op=Alu.mult)
        cos_t = ep_pool.tile([BP, 1], f32, tag="cost")
        nc.vector.tensor_reduce(out=cos_t[:, :], in_=ct_prod[:, :],
                                axis=mybir.AxisListType.X, op=Alu.add)
        loss_t = ep_pool.tile([BP, 1], f32, tag="losst")
        nc.vector.tensor_scalar(out=loss_t[:, :], in0=cos_t[:, :],
                                scalar1=MARGIN, scalar2=0.0,
                                op0=Alu.add, op1=Alu.max)

        # normal mask, count, sum of (1-cos) over normal agents
        notT = ep_pool.tile([BP, N], f32, tag="notT")
        nc.vector.tensor_scalar(out=notT[:, :], in0=onehot[:, :], scalar1=-1.0,
                                scalar2=1.0, op0=Alu.mult, op1=Alu.add)
        nmask = ep_pool.tile([BP, N], f32, tag="nmask")
        nc.vector.tensor_tensor(out=nmask[:, :], in0=mask_t[:, :],
                                in1=notT[:, :], op=Alu.mult)
        cnt = ep_pool.tile([BP, 1], f32, tag="cnt")
        nc.vector.tensor_reduce(out=cnt[:, :], in_=nmask[:, :],
                                axis=mybir.AxisListType.X, op=Alu.add)
        mc_prod = ep_pool.tile([BP, N], f32, tag="mcprod")
        nc.vector.tensor_tensor(out=mc_prod[:, :], in0=nmask[:, :],
                                in1=cos[:, :], op=Alu.mult)
        mc = ep_pool.tile([BP, 1], f32, tag="mc")
        nc.vector.tensor_reduce(out=mc[:, :], in_=mc_prod[:, :],
                                axis=mybir.AxisListType.X, op=Alu.add)
        so = ep_pool.tile([BP, 1], f32, tag="so")
        nc.vector.tensor_tensor(out=so[:, :], in0=cnt[:, :], in1=mc[:, :],
                                op=Alu.subtract)

        # loss_others = (cnt>0) ? so / max(cnt,1) : 0
        cnt1 = ep_pool.tile([BP, 1], f32, tag="cnt1")
        nc.vector.tensor_scalar(out=cnt1[:, :], in0=cnt[:, :], scalar1=1.0,
                                scalar2=None, op0=Alu.max)
        icnt = ep_pool.tile([BP, 1], f32, tag="icnt")
        nc.vector.reciprocal(out=icnt[:, :], in_=cnt1[:, :])
        gpos = ep_pool.tile([BP, 1], f32, tag="gpos")
        nc.vector.tensor_scalar(out=gpos[:, :], in0=cnt[:, :], scalar1=0.0,
                                scalar2=None, op0=Alu.is_gt)
        lo = ep_pool.tile([BP, 1], f32, tag="lo")
        nc.vector.tensor_tensor(out=lo[:, :], in0=so[:, :], in1=icnt[:, :],
                                op=Alu.mult)
        nc.vector.tensor_tensor(out=lo[:, :], in0=lo[:, :], in1=gpos[:, :],
                                op=Alu.mult)

        # per-sample loss, gated by validity
        pa = ep_pool.tile([BP, 1], f32, tag="pa")
        nc.vector.tensor_scalar(out=pa[:, :], in0=loss_t[:, :], scalar1=ALPHA,
                                scalar2=None, op0=Alu.mult)
        pb = ep_pool.tile([BP, 1], f32, tag="pb")
        nc.vector.tensor_scalar(out=pb[:, :], in0=lo[:, :],
                                scalar1=1.0 - ALPHA, scalar2=None, op0=Alu.mult)
        per = ep_pool.tile([BP, 1], f32, tag="per")
        nc.vector.tensor_tensor(out=per[:, :], in0=pa[:, :], in1=pb[:, :],
                                op=Alu.add)
        contrib = ep_pool.tile([BP, 1], f32, tag="contrib")
        nc.vector.tensor_tensor(out=contrib[:, :], in0=per[:, :],
                                in1=valid[:, :], op=Alu.mult)

        out_sb = ep_pool.tile([BP, 2], f32, tag="outsb")
        nc.vector.tensor_copy(out_sb[:, 0:1], contrib[:, :])
        nc.vector.tensor_copy(out_sb[:, 1:2], valid[:, :])
        nc.sync.dma_start(out=out_d[:, :], in_=out_sb[:, :])

    with tile.TileContext(nc) as tc, ExitStack() as ctx:
        pools = (
            ctx.enter_context(tc.tile_pool(name="mut", bufs=GROUPS)),
            ctx.enter_context(tc.tile_pool(name="heal", bufs=GROUPS)),
            ctx.enter_context(tc.tile_pool(name="t3", bufs=2)),
            ctx.enter_context(tc.tile_pool(name="stats", bufs=2)),
            ctx.enter_context(tc.tile_pool(name="epi", bufs=2)),
        )
        if reps == 1:
            emit_body(tc, ctx, pools)
        else:
            with tc.For_i(0, reps, 1):
                emit_body(tc, ctx, pools)

    nc.compile()
    return nc


def _get_nc(reps=1):
    key = ("nc", reps)
    if key not in _NC_CACHE:
        _NC_CACHE[key] = _build_nc(reps)
    return _NC_CACHE[key]


def _make_in_maps(inputs):
    import ml_dtypes

    bf = ml_dtypes.bfloat16
    mut = np.ascontiguousarray(
        np.asarray(inputs["emb_mut"], dtype=np.float32).astype(bf))
    heal = np.ascontiguousarray(
        np.asarray(inputs["emb_heal"], dtype=np.float32).astype(bf))
    idx_f = np.asarray(inputs["mistake_agent_idx"]).astype(np.float32).reshape(B, 1)
    mask_f = np.asarray(inputs["agent_mask"]).astype(np.float32).reshape(B, N)
    iota_f = np.ascontiguousarray(
        np.broadcast_to(np.arange(N, dtype=np.float32), (BP, N))
    )
    in_maps = []
    for c in range(NCORES):
        sl = slice(c * BP, (c + 1) * BP)
        in_maps.append({
            "emb_mut": mut[sl],
            "emb_heal": heal[sl],
            "idx_f": np.ascontiguousarray(idx_f[sl]),
            "mask_f": np.ascontiguousarray(mask_f[sl]),
            "iota_f": iota_f,
        })
    return in_maps


def run_spmd(inputs, trace=False, reps=1):
    """Run on all 8 cores; returns (final_scalar, BassKernelResults)."""
    from concourse.bass_utils import run_bass_kernel_spmd

    nc = _get_nc(reps)
    in_maps = _make_in_maps(inputs)
    res = run_bass_kernel_spmd(nc, in_maps, list(range(NCORES)), trace=trace)
    outs = np.stack([r["out"] for r in res.results])  # [8, 128, 2]
    total = outs[..., 0].sum(dtype=np.float64)
    count = outs[..., 1].sum(dtype=np.float64)
    val = np.float32(total / count) if count > 0 else np.float32(0.0)
    return val, res


def kernel(**inputs) -> np.ndarray:
    val, _ = run_spmd(inputs, trace=False)
    return val
